# revision 2
# baseline (speedup 1.0000x reference)
"""MultiHeadAttention forward on 8 Trainium2 NeuronCores (Bass/Tile).

Problem (hardcoded): B=2, S=2048, D=1024, H=16, HD=64.
  qkv = x @ w_qkv.T + b_qkv ; per-head attention with softmax(q k^T/8 + mask);
  out = values @ w_out.T + b_out.

Sharding: tensor-parallel over heads -- core c owns heads {2c, 2c+1}
(value dims 128c..128c+127).  Each core computes its 2 heads end-to-end and
a partial output projection; the host sums the 8 partials and adds the
bias constant (b_out + b_v @ w_out.T, exact because softmax rows sum to 1).

Device layout notes:
 - scores are computed TRANSPOSED (S^T[k,tq] = K^T.T @ Q^T per head) so the
   softmax exp can run on ScalarE straight out of PSUM and feed the AV
   matmul without any transposes.
 - V is projected in bf16 directly into [token, feat] layout with an extra
   ones column; the AV matmul (lhsT = [v|1]) then produces both values^T
   and the softmax denominator l in one pass.
 - vext carries 32 ones columns, so the AV output rows 64..95 all hold the
   denominator l; a 32x32 DVE block transpose makes l partition-parallel for
   a cheap reciprocal, a second transpose brings 1/l back as a row, and a
   K=1 PE matmul broadcasts it across partitions for the DVE normalize.
   (No SBUF->SBUF shuffle DMAs: tiny partition-scatter HWDGE transfers were
   observed to wedge the NeuronCore for the next NEFF execution.)
 - matmuls use float32r (full PE rate for moving dim >= 256, ~1e-4 rel err).
 - q/k score matmuls for the two heads are emitted back-to-back on disjoint
   PE row groups (partitions 0-63 / 64-127) so they run concurrently.
"""
import sys
if "/opt/trn_rl_repo" not in sys.path:
    sys.path.insert(0, "/opt/trn_rl_repo")
import numpy as np

B, S, D, H = 2, 2048, 1024, 16
HD = D // H           # 64
NCORES = 8
T = B * S             # 4096 tokens
NB = S // 512         # 4 tq blocks per batch
NCH = S // 128        # 16 kpos chunks per batch

_CACHE = {}


def build_nc(use_mask: bool, reps: int = 1, debug_dump: bool = False):
    """Build + compile the per-core Bass program (SPMD-identical)."""
    import concourse.bacc as bacc
    import concourse.tile as tile
    from concourse import mybir

    f32 = mybir.dt.float32
    f32r = mybir.dt.float32r
    bf16 = mybir.dt.bfloat16
    EXP = mybir.ActivationFunctionType.Exp
    MULT = mybir.AluOpType.mult

    nc = bacc.Bacc("TRN2", target_bir_lowering=False, debug=False,
                   num_devices=NCORES)

    xT = nc.dram_tensor("xT", (D, T), f32r, kind="ExternalInput")
    xTb = nc.dram_tensor("xTb", (D, T), bf16, kind="ExternalInput")
    wqkT = nc.dram_tensor("wqkT", (D, 256), f32r, kind="ExternalInput")
    bqk = nc.dram_tensor("bqk", (128, 2), f32, kind="ExternalInput")
    wvT = nc.dram_tensor("wvT", (D, 128), bf16, kind="ExternalInput")
    woT = nc.dram_tensor("woT", (64, 2, D), f32r, kind="ExternalInput")
    onesd = nc.dram_tensor("onesd", (1, 65), f32r, kind="ExternalInput")
    if use_mask:
        maskT = nc.dram_tensor("maskT", (B, S, S), f32r, kind="ExternalInput")
        ident = nc.dram_tensor("ident", (128, 128), f32r, kind="ExternalInput")
    out = nc.dram_tensor("out", (T, D), f32, kind="ExternalOutput")
    import os
    _dbg = set(os.environ.get("DBG", "qkt,vals,rl").split(",")) if debug_dump else set()
    if "qkt" in _dbg:
        d_qkt = nc.dram_tensor("d_qkt", (128, 2, T), f32r, kind="ExternalOutput")
    if "vals" in _dbg:
        d_vals0 = nc.dram_tensor("d_vals0", (64, T), f32r, kind="ExternalOutput")
        d_vals1 = nc.dram_tensor("d_vals1", (64, T), f32r, kind="ExternalOutput")
    if "rl" in _dbg:
        d_rl = nc.dram_tensor("d_rl", (B * 2 * NB, 512), f32r, kind="ExternalOutput")

    with tile.TileContext(nc) as tc:
        with tc.tile_pool(name="sbp", bufs=1) as sbp, \
             tc.tile_pool(name="xtp", bufs=10) as xtp, \
             tc.tile_pool(name="xtbp", bufs=10) as xtbp, \
             tc.tile_pool(name="ptp", bufs=4) as ptp, \
             tc.tile_pool(name="lrp", bufs=2) as lrp, \
             tc.tile_pool(name="otp", bufs=4) as otp, \
             tc.tile_pool(name="mkp", bufs=4) as mkp, \
             tc.tile_pool(name="mmp", bufs=2, space="PSUM") as mmp, \
             tc.tile_pool(name="scp", bufs=2, space="PSUM") as scp, \
             tc.tile_pool(name="avp", bufs=2, space="PSUM") as avp:

            # --- persistent SBUF tensors ---
            qkt = sbp.tile([128, 2, T], f32r, name="qkt")        # [feat, {q,k}, tok]
            vext = sbp.tile([128, B, 2, NCH, HD + 32], bf16, name="vext")
            valsT0 = sbp.tile([64, T], f32r, name="valsT0")
            valsT1 = sbp.tile([64, T], f32r, name="valsT1")
            wqk_sb = sbp.tile([128, 8, 256], f32r, name="wqk_sb")
            wv_sb = sbp.tile([128, 8, 128], bf16, name="wv_sb")
            wo_sb = sbp.tile([64, 2, D], f32r, name="wo_sb")
            bqk_sb = sbp.tile([128, 2], f32, name="bqk_sb")
            ones_sb = sbp.tile([65, 65], f32r, name="ones_sb")
            if use_mask:
                id_sb = sbp.tile([128, 128], f32r, name="id_sb")
                nc.sync.dma_start(id_sb, ident[:, :])

            for c in range(8):
                nc.sync.dma_start(wqk_sb[:, c, :], wqkT[128 * c:128 * c + 128, :])
                nc.sync.dma_start(wv_sb[:, c, :], wvT[128 * c:128 * c + 128, :])
            nc.sync.dma_start(wo_sb, woT[:, :, :])
            nc.sync.dma_start(bqk_sb, bqk[:, :])
            for _op in range(65):
                nc.sync.dma_start(ones_sb[_op:_op + 1, :], onesd[:, :])
            nc.vector.memset(vext[:, :, :, :, HD:HD + 32], 1.0)

            for rep in range(reps):
                # ================= Phase A: projections =================
                for tb in range(8):          # 512-token blocks over all 4096
                    xts, xtbs = [], []
                    for c in range(8):       # D chunks
                        xt = xtp.tile([128, 512], f32r, tag="xt",
                                      name=f"xt_{rep}_{tb}_{c}")
                        nc.sync.dma_start(
                            xt, xT[128 * c:128 * c + 128, 512 * tb:512 * tb + 512])
                        xts.append(xt)
                        xtb_t = xtbp.tile([128, 512], bf16, tag="xtb",
                                          name=f"xtb_{rep}_{tb}_{c}")
                        nc.sync.dma_start(
                            xtb_t, xTb[128 * c:128 * c + 128, 512 * tb:512 * tb + 512])
                        xtbs.append(xtb_t)
                    # q/k projections: out [feat 128, tok 512]
                    for m in range(2):
                        acc = mmp.tile([128, 512], f32, tag="mm",
                                       name=f"qk_{rep}_{tb}_{m}")
                        for c in range(8):
                            nc.tensor.matmul(
                                acc, wqk_sb[:, c, 128 * m:128 * m + 128], xts[c],
                                start=(c == 0), stop=(c == 7))
                        nc.vector.tensor_scalar_add(
                            qkt[:, m, 512 * tb:512 * tb + 512], acc,
                            bqk_sb[:, m:m + 1])
                    # v projection: out [tok 128, vfeat 128] (bf16 inputs)
                    for u in range(4):
                        tt = 4 * tb + u
                        b, cc = tt // NCH, tt % NCH
                        vp = mmp.tile([128, 128], f32, tag="mm",
                                      name=f"vp_{rep}_{tt}")
                        for c in range(8):
                            nc.tensor.matmul(
                                vp, xtbs[c][:, 128 * u:128 * u + 128],
                                wv_sb[:, c, :], start=(c == 0), stop=(c == 7))
                        nc.vector.tensor_copy(vext[:, b, 0, cc, 0:HD], vp[:, 0:64])
                        nc.vector.tensor_copy(vext[:, b, 1, cc, 0:HD], vp[:, 64:128])

                # ============ Phase B: attention, + Phase C per batch ============
                for b in range(B):
                    for tqb in range(NB):
                        tq0 = S * b + 512 * tqb
                        q_aps = [qkt[64 * h:64 * h + 64, 0, tq0:tq0 + 512]
                                 for h in range(2)]
                        avs = [avp.tile([128, 512], f32, tag="av",
                                        name=f"av_{rep}_{b}_{h}_{tqb}")
                               for h in range(2)]
                        for c in range(NCH):
                            # one sc tile holds chunk c for BOTH heads; the two
                            # score matmuls hit disjoint PE row groups (d 0-63 /
                            # 64-127) and run concurrently.
                            sc = scp.tile([128, 1024], f32, tag="sc",
                                          name=f"sc_{rep}_{b}_{tqb}_{c}")
                            for h in range(2):
                                k_ap = qkt[64 * h:64 * h + 64, 1,
                                           S * b + 128 * c:S * b + 128 * c + 128]
                                nc.tensor.matmul(
                                    sc[:, 512 * h:512 * h + 512], k_ap, q_aps[h],
                                    start=True, stop=(not use_mask))
                            if use_mask:
                                mt = mkp.tile([128, 512], f32r, tag="mk",
                                              name=f"mk_{rep}_{b}_{tqb}_{c}")
                                nc.sync.dma_start(
                                    mt, maskT[b, 128 * c:128 * c + 128,
                                              512 * tqb:512 * tqb + 512])
                                for h in range(2):
                                    nc.tensor.matmul(
                                        sc[:, 512 * h:512 * h + 512], id_sb, mt,
                                        start=False, stop=True)
                            pt = ptp.tile([128, 1024], bf16, tag="pt",
                                          name=f"pt_{rep}_{b}_{tqb}_{c}")
                            nc.scalar.activation(pt, sc, EXP)
                            for h in range(2):
                                nc.tensor.matmul(
                                    avs[h][0:96, :], vext[:, b, h, c, :],
                                    pt[:, 512 * h:512 * h + 512],
                                    start=(c == 0), stop=(c == NCH - 1))
                        # --- normalize: values^T[:, tq] = av[0:64] / l ---
                        # av rows 64..95 all hold l (32 ones columns in vext);
                        # 32x32 DVE block transposes give a partition-parallel
                        # reciprocal without any DMA.
                        for h in range(2):
                            av = avs[h]
                            ls = lrp.tile([96, 512], f32, tag="ls",
                                          name=f"ls_{rep}_{b}_{h}_{tqb}")
                            nc.vector.tensor_copy(ls[64:96, :], av[64:96, :])
                            lt = lrp.tile([96, 512], f32, tag="lt",
                                          name=f"lt_{rep}_{b}_{h}_{tqb}")
                            nc.vector.transpose(lt[64:96, :], ls[64:96, :])
                            rlp = lrp.tile([96, 512], f32, tag="rlp",
                                           name=f"rlp_{rep}_{b}_{h}_{tqb}")
                            lt3 = lt[64:96, :].rearrange(
                                "p (a b) -> p a b", b=32)[:, :, 0:1]
                            rlp3 = rlp[64:96, :].rearrange(
                                "p (a b) -> p a b", b=32)[:, :, 0:1]
                            nc.vector.reciprocal(rlp3, lt3)
                            rlrowf = lrp.tile([96, 512], f32, tag="rlrowf",
                                              name=f"rlrowf_{rep}_{b}_{h}_{tqb}")
                            nc.vector.transpose(rlrowf[64:96, :], rlp[64:96, :])
                            rlrow = lrp.tile([65, 512], f32r, tag="rlrow",
                                             name=f"rlrow_{rep}_{b}_{h}_{tqb}")
                            nc.vector.tensor_copy(rlrow[64:65, :],
                                                  rlrowf[64:65, :])
                            if "rl" in _dbg and rep == 0:
                                u = (b * 2 + h) * NB + tqb
                                nc.sync.dma_start(d_rl[u:u + 1, :],
                                                  rlrow[64:65, :])
                            bc = mmp.tile([64, 512], f32, tag="mm",
                                          name=f"bc_{rep}_{b}_{h}_{tqb}")
                            nc.tensor.matmul(bc, ones_sb[64:65, 0:64],
                                             rlrow[64:65, :],
                                             start=True, stop=True)
                            bcs = lrp.tile([64, 512], f32, tag="bcs",
                                           name=f"bcs_{rep}_{b}_{h}_{tqb}")
                            nc.vector.tensor_copy(bcs, bc)
                            vt = valsT0 if h == 0 else valsT1
                            nc.vector.tensor_tensor(
                                vt[:, tq0:tq0 + 512], av[0:64, :], bcs, MULT)
                        # ---- Phase C interleaved: this tq-block's out rows ----
                        for nb in range(2):
                            for u in range(4):
                                t0 = tq0 + 128 * u
                                op = mmp.tile([128, 512], f32, tag="mm",
                                              name=f"op_{rep}_{b}_{tqb}_{nb}_{u}")
                                nc.tensor.matmul(
                                    op, valsT0[:, t0:t0 + 128],
                                    wo_sb[:, 0, 512 * nb:512 * nb + 512],
                                    start=True, stop=False)
                                nc.tensor.matmul(
                                    op, valsT1[:, t0:t0 + 128],
                                    wo_sb[:, 1, 512 * nb:512 * nb + 512],
                                    start=False, stop=True)
                                ot = otp.tile([128, 512], f32, tag="ot",
                                              name=f"ot_{rep}_{b}_{tqb}_{nb}_{u}")
                                nc.vector.tensor_copy(ot, op)
                                nc.sync.dma_start(
                                    out[t0:t0 + 128, 512 * nb:512 * nb + 512], ot)
            if "qkt" in _dbg:
                nc.sync.dma_start(d_qkt[:, :, :], qkt)
            if "vals" in _dbg:
                nc.sync.dma_start(d_vals0[:, :], valsT0)
                nc.sync.dma_start(d_vals1[:, :], valsT1)
    nc.compile()
    return nc


def make_in_maps(mha_x, self_mask, w_qkv, b_qkv, w_out, b_out, use_mask):
    """Host-side sharding / layout prep. Returns (in_maps, host_bias)."""
    import ml_dtypes
    bf = np.dtype(ml_dtypes.bfloat16)
    x = np.asarray(mha_x, np.float32).reshape(T, D)
    xT_np = np.ascontiguousarray(x.T)                   # [D, T]
    xTb_np = np.ascontiguousarray(xT_np.astype(bf))
    scale = 1.0 / np.sqrt(np.float32(HD))               # 1/8
    wqkv = np.asarray(w_qkv, np.float32)
    bqkv = np.asarray(b_qkv, np.float32)
    wout = np.asarray(w_out, np.float32)
    bout = np.asarray(b_out, np.float32)

    # reference packs w_qkv rows as [H, (q,k,v), HD]: head h's q rows are
    # wqkv[192h:192h+64], k rows +64, v rows +128.
    wq_rows = lambda h: wqkv[192 * h:192 * h + 64, :]
    wk_rows = lambda h: wqkv[192 * h + 64:192 * h + 128, :]
    wv_rows = lambda h: wqkv[192 * h + 128:192 * h + 192, :]
    bq_of = lambda h: bqkv[192 * h:192 * h + 64]
    bk_of = lambda h: bqkv[192 * h + 64:192 * h + 128]
    bv_of = lambda h: bqkv[192 * h + 128:192 * h + 192]

    in_maps = []
    for c in range(NCORES):
        h0, h1 = 2 * c, 2 * c + 1
        wq = np.concatenate([wq_rows(h0), wq_rows(h1)], 0) * scale
        wk = np.concatenate([wk_rows(h0), wk_rows(h1)], 0)
        wv = np.concatenate([wv_rows(h0), wv_rows(h1)], 0)
        m = {
            "xT": xT_np,
            "xTb": xTb_np,
            "wqkT": np.ascontiguousarray(np.concatenate([wq, wk], 0).T),
            "bqk": np.ascontiguousarray(
                np.stack([np.concatenate([bq_of(h0), bq_of(h1)]) * scale,
                          np.concatenate([bk_of(h0), bk_of(h1)])], 1)),
            "wvT": np.ascontiguousarray(wv.T.astype(bf)),
            "woT": np.ascontiguousarray(
                wout[:, 128 * c:128 * c + 128].T.reshape(2, 64, D).transpose(1, 0, 2)),
            "onesd": np.ones((1, 65), np.float32),
        }
        if use_mask:
            m["maskT"] = np.ascontiguousarray(
                np.asarray(self_mask, np.float32).transpose(0, 2, 1))
            m["ident"] = np.eye(128, dtype=np.float32)
        in_maps.append(m)

    b_v_full = np.concatenate([bv_of(h) for h in range(H)])
    host_bias = b_v_full @ wout.T + bout                # [D], exact
    return in_maps, host_bias


def combine_outputs(results, host_bias):
    """results: list of per-core {"out": [T,D]} dicts."""
    acc = np.zeros((T, D), np.float32)
    for c in range(NCORES):
        acc += results[c]["out"]
    acc += host_bias[None, :]
    return acc.reshape(B, S, D)


def kernel(**inputs):
    from concourse.bass_utils import run_bass_kernel_spmd
    self_mask = np.asarray(inputs["self_mask"], np.float32)
    use_mask = bool(np.any(self_mask))
    key = ("nc", use_mask)
    if key not in _CACHE:
        _CACHE[key] = build_nc(use_mask)
    nc = _CACHE[key]
    in_maps, host_bias = make_in_maps(
        inputs["mha_x"], self_mask, inputs["w_qkv"], inputs["b_qkv"],
        inputs["w_out"], inputs["b_out"], use_mask)
    res = run_bass_kernel_spmd(nc, in_maps, core_ids=list(range(NCORES)))
    return combine_outputs(res.results, host_bias)

